# revision 1
# baseline (speedup 1.0000x reference)
"""MoBA (Mixture of Block Attention) Trainium2 Bass kernel.

Problem: B=1, S=2048, D=2048, H=16 heads (d=128), chunk=256, topk=4.
Sharding: 2 heads per core across 8 cores (tensor-parallel on H).
Each core computes q/k/v projections for its 2 heads (fp32r matmuls to
keep the block-gate at ~FP22 precision), RoPE, per-head block gating
(top-4 of 8 chunks), sparse-masked causal attention computed in
transposed score layout (scoresT[key, q]), RMSNorm, and a partial
output projection o_norm @ Wo[:, head_slice].T.  The host sums the 8
partial outputs.

The block mask is applied by accumulating a K=8 one-hot matmul of the
transposed mask rows into the score PSUM before the exp, so the exp is
a single bias-free activation per score tile and the probabilities come
out already transposed for the PV matmul (no on-chip transposes of the
probability matrix at all).

Self-contained: hardcodes all shapes; builds and caches one Bass/Tile
program, runs it SPMD on cores 0-7 via run_bass_kernel_spmd.
"""

import math
from contextlib import ExitStack

import numpy as np
import ml_dtypes

S = 2048
D = 2048
H = 16
DH = 128          # head dim
CHUNK = 256
NBLK = S // CHUNK  # 8
NCORES = 8
HLOC = H // NCORES  # 2 heads per core
FLOC = HLOC * DH    # 256 features per core
P = 128
NT = S // P         # 16 query chunks of 128
SM_SCALE = DH ** -0.5
NEG = -1.0e30
POS = 1.0e30
NEGBIG = -3.0e30
EPS = 1e-6
THETA = 10000.0

# packed offsets of the per-key-chunk probsT pieces: piece c holds
# queries [128c, 2048) -> width 2048-128c
PB_OFF = [0] * (NT + 1)
for _c in range(NT):
    PB_OFF[_c + 1] = PB_OFF[_c] + (S - P * _c)
PB_TOT = PB_OFF[NT]  # 17408

_CACHE = {}


def _build_program():
    import concourse.bacc as bacc
    import concourse.tile as tile
    from concourse import mybir

    f32 = mybir.dt.float32
    f32r = mybir.dt.float32r
    bf16 = mybir.dt.bfloat16
    X = mybir.AxisListType
    AF = mybir.ActivationFunctionType
    OP = mybir.AluOpType

    nc = bacc.Bacc("TRN2", target_bir_lowering=False, debug=False)

    def din(name, shape, dt):
        return nc.dram_tensor(name, shape, dt, kind="ExternalInput").ap()

    hT_d = din("hT", [D, S], f32r)          # hidden transposed [D, S]
    wq_d = din("wq", [D, FLOC], f32r)       # Wq[hs,:].T
    wk_d = din("wk", [D, FLOC], f32r)
    wv_d = din("wv", [D, FLOC], f32r)
    wo_d = din("wo", [FLOC, D], bf16)      # (Wo[:,hs] * w).T
    cos2_d = din("cos2", [P, S], f32)      # [cosT; cosT]
    sin2_d = din("sin2", [P, S], f32)      # [-sinT; sinT]
    triT_d = din("triT", [P, P], f32)      # triT[i,j] = 0 if i<=j else NEG
    id_d = din("ident", [P, P], bf16)      # identity for PE transpose (bf16)
    id32_d = din("id32", [P, P], f32)      # identity (f32)
    pswap_d = din("pswap", [P, P], f32r)    # half-swap permutation
    oneh_d = din("oneh", [NBLK, NBLK * P], f32r)  # oneh[k, b*128+m] = (k==b)
    mulM_d = din("mulM", [P, P], f32)      # gate mult mask  [*, t*8+n]
    addM_d = din("addM", [P, P], f32)      # gate add mask
    out_d = nc.dram_tensor("out", [S, D], f32, kind="ExternalOutput").ap()

    with tile.TileContext(nc) as tc:
        with ExitStack() as ctx:
            const_pool = ctx.enter_context(tc.tile_pool(name="consts", bufs=1))
            qkv_pool = ctx.enter_context(tc.tile_pool(name="qkv", bufs=1))
            tk_pool = ctx.enter_context(tc.tile_pool(name="topk", bufs=1))
            ps_big = ctx.enter_context(
                tc.tile_pool(name="psbig", bufs=4, space="PSUM"))
            ps_wo = ps_big
            ps_po = ctx.enter_context(
                tc.tile_pool(name="pspo", bufs=2, space="PSUM"))
            ps_rs = ctx.enter_context(
                tc.tile_pool(name="psrs", bufs=2, space="PSUM"))

            # ---- constants ----
            cos2_t = const_pool.tile([P, S], f32)
            sin2_t = const_pool.tile([P, S], f32)
            triT_t = const_pool.tile([P, P], f32)
            id_t = const_pool.tile([P, P], bf16)
            id32_t = const_pool.tile([P, P], f32)
            pswap_t = const_pool.tile([P, P], f32r)
            oneh_t = const_pool.tile([NBLK, NBLK, P], f32r)
            mulM_t = const_pool.tile([P, P], f32)
            addM_t = const_pool.tile([P, P], f32)
            negbig_t = const_pool.tile([P, 2 * P], f32)
            zero_t = const_pool.tile([P, 1], f32)
            onescol_t = const_pool.tile([P, 1], bf16)
            wo_t = const_pool.tile([P, HLOC, D], bf16)
            nc.scalar.dma_start(cos2_t[:], cos2_d)
            nc.scalar.dma_start(sin2_t[:], sin2_d)
            nc.scalar.dma_start(triT_t[:], triT_d)
            nc.scalar.dma_start(id_t[:], id_d)
            nc.scalar.dma_start(id32_t[:], id32_d)
            nc.scalar.dma_start(pswap_t[:], pswap_d)
            nc.scalar.dma_start(oneh_t[:], oneh_d.rearrange("k (b m) -> k b m", m=P))
            nc.scalar.dma_start(mulM_t[:], mulM_d)
            nc.scalar.dma_start(addM_t[:], addM_d)
            nc.scalar.dma_start(wo_t[:], wo_d.rearrange("(fc p) j -> p fc j", p=P))
            nc.vector.memset(negbig_t[:], NEGBIG)
            nc.vector.memset(zero_t[:], 0.0)
            nc.vector.memset(onescol_t[:], 1.0)

            # ---- persistent activations ----
            qT = qkv_pool.tile([P, HLOC, S], f32r)    # roped q, [d, head, s]
            kT = qkv_pool.tile([P, HLOC, S], f32r)
            v_sb = qkv_pool.tile([P, NT, FLOC], bf16)
            O_sb = qkv_pool.tile([P, HLOC, NT, DH], f32)
            onT_sb = qkv_pool.tile([P, HLOC, NT, P], bf16)
            km_t = qkv_pool.tile([P, HLOC, NBLK], f32r)
            gate_sb = qkv_pool.tile([P, HLOC, NT * NBLK], f32)
            M_sb = qkv_pool.tile([P, HLOC, NT * NBLK], f32)
            MTf_sb = qkv_pool.tile([NBLK, HLOC, S], f32r)  # MTf[n,h,q]=M[q,n]

            hT_r = hT_d.rearrange("(dc p) s -> p dc s", p=P)

            # ============ phase A: projections + rope (per 256-token tile) ===
            with ExitStack() as actx:
                w_pool = actx.enter_context(tc.tile_pool(name="weights", bufs=1))
                hid_pool = actx.enter_context(tc.tile_pool(name="hid", bufs=2))
                rsc_pool = actx.enter_context(tc.tile_pool(name="ropes", bufs=2))

                wq_t = w_pool.tile([P, 16, FLOC], f32r)
                wk_t = w_pool.tile([P, 16, FLOC], f32r)
                wv_t = w_pool.tile([P, 16, FLOC], f32r)
                # order matters: sync-HWDGE DMAs drain FIFO per engine, so
                # issue (wq, first hidden tile) before wk/wv to start the
                # first q-projection matmuls as early as possible.
                wq_r = wq_d.rearrange("(dc p) f -> p dc f", p=P)
                ht0 = hid_pool.tile([P, 16, 256], f32r, tag="hid", name="ht0")
                nc.sync.dma_start(wq_t[:, 0:8, :], wq_r[:, 0:8, :])
                nc.sync.dma_start(ht0[:, 0:8, :], hT_r[:, 0:8, 0:256])
                nc.sync.dma_start(wq_t[:, 8:16, :], wq_r[:, 8:16, :])
                nc.sync.dma_start(ht0[:, 8:16, :], hT_r[:, 8:16, 0:256])
                nc.sync.dma_start(wk_t[:], wk_d.rearrange("(dc p) f -> p dc f", p=P))
                nc.sync.dma_start(wv_t[:], wv_d.rearrange("(dc p) f -> p dc f", p=P))

                deferred_v = []
                for st in range(8):
                    sl = slice(st * 256, (st + 1) * 256)
                    if st == 0:
                        ht = ht0
                    else:
                        ht = hid_pool.tile([P, 16, 256], f32r, tag="hid")
                        nc.sync.dma_start(ht[:], hT_r[:, :, sl])
                    for wt, dst in ((wq_t, qT), (wk_t, kT)):
                        for fc in range(HLOC):
                            pq = ps_big.tile([P, 256], f32, tag="big")
                            for dc in range(16):
                                nc.tensor.matmul(
                                    pq[:],
                                    lhsT=wt[:, dc, fc * P:(fc + 1) * P],
                                    rhs=ht[:, dc, :],
                                    start=(dc == 0),
                                    stop=(dc == 15),
                                )
                            nc.vector.tensor_copy(dst[:, fc, sl], pq[:])
                    if st < 6:
                        for sc in range(2):
                            pv = ps_po.tile([P, FLOC], f32, tag="po")
                            for dc in range(16):
                                nc.tensor.matmul(
                                    pv[:],
                                    lhsT=ht[:, dc, sc * P:(sc + 1) * P],
                                    rhs=wv_t[:, dc, :],
                                    start=(dc == 0),
                                    stop=(dc == 15),
                                )
                            nc.vector.tensor_copy(v_sb[:, st * 2 + sc, :], pv[:])
                    else:
                        deferred_v.append((st, ht))
                    # rope on this tile (after q/k written)
                    for dst in (qT, kT):
                        for hh in range(HLOC):
                            psw = ps_big.tile([P, 256], f32, tag="big")
                            nc.tensor.matmul(
                                psw[:],
                                lhsT=pswap_t[:],
                                rhs=dst[:, hh, sl],
                                start=True, stop=True,
                            )
                            rs_scr = rsc_pool.tile([P, 256], f32, tag="rope")
                            nc.vector.tensor_mul(rs_scr[:], psw[:], sin2_t[:, sl])
                            nc.vector.tensor_mul(dst[:, hh, sl], dst[:, hh, sl],
                                                 cos2_t[:, sl])
                            nc.vector.tensor_add(dst[:, hh, sl], dst[:, hh, sl],
                                                 rs_scr[:])
                    # incremental block means: S-tile == one 256 chunk
                    # (fp32 internal accumulation; only the final write is
                    # rounded to fp32r for the gate matmul)
                    with nc.allow_low_precision(reason="km written as fp32r"):
                        for hh in range(HLOC):
                            nc.vector.reduce_sum(km_t[:, hh, st:st + 1],
                                                 kT[:, hh, sl], axis=X.X)

                # ---- gate ----
                for hh in range(HLOC):
                    pg = ps_rs.tile([P, P], f32, tag="rs")
                    for t in range(NT):
                        nc.tensor.matmul(
                            pg[:, t * NBLK:(t + 1) * NBLK],
                            lhsT=qT[:, hh, t * P:(t + 1) * P],
                            rhs=km_t[:, hh, :],
                            start=True, stop=True,
                        )
                    nc.vector.tensor_mul(gate_sb[:, hh, :], pg[:], mulM_t[:])
                    nc.vector.tensor_add(gate_sb[:, hh, :], gate_sb[:, hh, :],
                                         addM_t[:])

                # ---- top-4 selection -> additive mask M_sb ----
                G = HLOC * NT  # 32 groups of 8 blocks
                gw_t = tk_pool.tile([P, G * NBLK], f32)
                lt_t = tk_pool.tile([P, G * NBLK], mybir.dt.int32)
                m_t = tk_pool.tile([P, G], f32)
                gate_f = gate_sb[:].rearrange("p h g -> p (h g)")
                gw_v = gw_t[:].rearrange("p (g n) -> p g n", n=NBLK)
                lt_v = lt_t[:].rearrange("p (g n) -> p g n", n=NBLK)
                nc.vector.tensor_copy(gw_t[:], gate_f)
                for _ in range(3):
                    nc.vector.reduce_max(m_t[:], gw_v, axis=X.X)
                    mb = m_t[:].rearrange("p (g o) -> p g o", o=1).to_broadcast(
                        (P, G, NBLK))
                    nc.vector.tensor_tensor(lt_v, gw_v, mb, op=OP.is_ge)
                    nc.vector.copy_predicated(gw_t[:], lt_t[:], negbig_t[:])
                nc.vector.reduce_max(m_t[:], gw_v, axis=X.X)
                nc.vector.tensor_scalar_max(m_t[:], m_t[:], -1.0e29)
                mb = m_t[:].rearrange("p (g o) -> p g o", o=1).to_broadcast(
                    (P, G, NBLK))
                gate_v = gate_sb[:].rearrange("p h (t n) -> p (h t) n", n=NBLK)
                M_v = M_sb[:].rearrange("p h (t n) -> p (h t) n", n=NBLK)
                nc.vector.tensor_tensor(M_v, gate_v, mb, op=OP.is_ge)
                nc.vector.tensor_scalar(
                    M_sb[:].rearrange("p h g -> p (h g)"),
                    M_sb[:].rearrange("p h g -> p (h g)"),
                    1.0, POS, op0=OP.subtract, op1=OP.mult,
                )
                # transpose M -> MTf[n, h, 128t+p] = M[q=128t+p, h, (t,n)]
                for hh in range(HLOC):
                    for t in range(NT):
                        pmt = ps_rs.tile([NBLK, P], f32, tag="rs")
                        nc.tensor.transpose(
                            pmt[:], M_sb[:, hh, t * NBLK:(t + 1) * NBLK],
                            id32_t[:])
                        nc.vector.tensor_copy(
                            MTf_sb[:, hh, t * P:(t + 1) * P], pmt[:])

                # deferred v projections overlap the gating chain above
                for st_, ht_ in deferred_v:
                    for sc in range(2):
                        pv = ps_po.tile([P, FLOC], f32, tag="po")
                        for dc in range(16):
                            nc.tensor.matmul(
                                pv[:],
                                lhsT=ht_[:, dc, sc * P:(sc + 1) * P],
                                rhs=wv_t[:, dc, :],
                                start=(dc == 0),
                                stop=(dc == 15),
                            )
                        nc.vector.tensor_copy(v_sb[:, st_ * 2 + sc, :], pv[:])

            # ============ phase B: gating, attention, norm, output ===========
            with ExitStack() as bctx:
                att_pool = bctx.enter_context(tc.tile_pool(name="att", bufs=2))
                pb_pool = bctx.enter_context(tc.tile_pool(name="probs", bufs=1))
                orow_pool = bctx.enter_context(tc.tile_pool(name="orow", bufs=2))

                # ---- attention pass 1: scoresT -> masked exp -> probsT ----
                pb_tiles = [[pb_pool.tile([P, S - P * _c], bf16,
                                          name=f"pb{_h}_{_c}",
                                          tag=f"pb{_h}_{_c}")
                             for _c in range(NT)]
                            for _h in range(HLOC)]
                def emit_pass2(t):
                    for hh in range(HLOC):
                        po = ps_po.tile([P, DH], f32, tag="po", name="po_e")
                        prs = ps_rs.tile([P, 1], f32, tag="rs", name="prs_e")
                        for c2 in range(t + 1):
                            lhs = pb_tiles[hh][c2][:, P * (t - c2):
                                                   P * (t - c2) + P]
                            nc.tensor.matmul(
                                po[:], lhsT=lhs,
                                rhs=v_sb[:, c2, hh * DH:(hh + 1) * DH],
                                start=(c2 == 0), stop=(c2 == t),
                            )
                            nc.tensor.matmul(
                                prs[:], lhsT=lhs, rhs=onescol_t[:],
                                start=(c2 == 0), stop=(c2 == t),
                            )
                        nc.vector.tensor_copy(O_sb[:, hh, t, :], po[:])
                        ssa = att_pool.tile([P, 1], f32, tag="ssa", name="ssa_e")
                        ssb = att_pool.tile([P, 1], f32, tag="ssb", name="ssb_e")
                        sq = att_pool.tile([P, DH], f32, tag="sq", name="sq_e")
                        nc.scalar.activation(
                            sq[:], O_sb[:, hh, t, :], AF.Square,
                            accum_out=ssa[:])
                        rsum_sb = att_pool.tile([P, 1], f32, tag="rsum",
                                                name="rsum_e")
                        nc.vector.tensor_copy(rsum_sb[:], prs[:])
                        nc.vector.tensor_tensor(ssb[:], rsum_sb[:], rsum_sb[:],
                                                op=OP.mult)
                        nc.vector.tensor_scalar_mul(ssb[:], ssb[:], EPS)
                        sS = att_pool.tile([P, 1], f32, tag="sS", name="sS_e")
                        nc.scalar.activation(sS[:], ssa[:], AF.Sqrt,
                                             bias=ssb[:], scale=1.0 / DH)
                        nc.vector.reciprocal(sS[:], sS[:])
                        onp = att_pool.tile([P, DH], bf16, tag="onp", name="onp_e")
                        nc.vector.tensor_scalar_mul(
                            onp[:], O_sb[:, hh, t, :], sS[:])
                        ptr = ps_po.tile([P, P], bf16, tag="po", name="ptr_e")
                        nc.tensor.transpose(ptr[:], onp[:], id_t[:])
                        if hh % 2 == 0:
                            nc.vector.tensor_copy(onT_sb[:, hh, t, :], ptr[:])
                        else:
                            nc.scalar.copy(onT_sb[:, hh, t, :], ptr[:])
                    orow = orow_pool.tile([P, D], f32, tag="orow", name="orow_e")
                    for nt in range(4):
                        pso = ps_wo.tile([P, 512], f32, tag="big", name="pso_e")
                        for hh in range(HLOC):
                            nc.tensor.matmul(
                                pso[:],
                                lhsT=onT_sb[:, hh, t, :],
                                rhs=wo_t[:, hh, nt * 512:(nt + 1) * 512],
                                start=(hh == 0), stop=(hh == HLOC - 1),
                            )
                        if nt % 2 == 0:
                            nc.vector.tensor_copy(
                                orow[:, nt * 512:(nt + 1) * 512], pso[:])
                        else:
                            nc.scalar.copy(
                                orow[:, nt * 512:(nt + 1) * 512], pso[:])
                    nc.sync.dma_start(out_d[t * P:(t + 1) * P, :], orow[:])

                for c in range(NT):
                    b = c // 2
                    q0 = P * c
                    for hh in range(HLOC):
                        pbp = pb_tiles[hh][c]
                        for j, qs in enumerate(range(q0, S, 512)):
                            w = min(512, S - qs)
                            psc = ps_big.tile([P, 512], f32, tag="big")
                            nc.tensor.matmul(
                                psc[:, :w],
                                lhsT=kT[:, hh, c * P:(c + 1) * P],
                                rhs=qT[:, hh, qs:qs + w],
                                start=True, stop=False,
                            )
                            nc.tensor.matmul(
                                psc[:, :w],
                                lhsT=oneh_t[:, b, :],
                                rhs=MTf_sb[:, hh, qs:qs + w],
                                start=False, stop=True,
                            )
                            if j == 0:
                                nc.vector.tensor_add(
                                    psc[:, :P], psc[:, :P], triT_t[:])
                            nc.scalar.activation(
                                pbp[:, qs - q0:qs - q0 + w], psc[:, :w], AF.Exp,
                                bias=zero_t[:], scale=SM_SCALE)



                for t_ in range(NT):
                    emit_pass2(t_)

    nc.compile()
    return nc


def _host_inputs(hidden, Wq, Wk, Wv, Wo, o_norm_w):
    """Build the per-core input maps (host-side sharding + prep)."""
    def fp22_round(x):
        """Round fp32 mantissa to 13 bits (FP22, round-half-to-even) so the
        fp32r TensorEngine path sees exactly these values."""
        u = np.ascontiguousarray(x, dtype=np.float32).view(np.uint32)
        lsb = (u >> np.uint32(10)) & np.uint32(1)
        r = (u + np.uint32(0x1FF) + lsb) & np.uint32(0xFFFFFC00)
        return r.view(np.float32)

    h = np.ascontiguousarray(np.asarray(hidden, dtype=np.float32).reshape(S, D))
    Wq = fp22_round(np.asarray(Wq, dtype=np.float32))
    Wk = fp22_round(np.asarray(Wk, dtype=np.float32))
    Wv = fp22_round(np.asarray(Wv, dtype=np.float32))
    Wo = np.asarray(Wo, dtype=np.float32)
    w = np.asarray(o_norm_w, dtype=np.float32)

    hT = fp22_round(np.ascontiguousarray(h.T))

    pos = np.arange(S, dtype=np.float64)
    inv = 1.0 / (THETA ** (np.arange(0, DH, 2, dtype=np.float64) / DH))
    fr = pos[:, None] * inv[None, :]                # [S, 64]
    cosT = np.cos(fr).T.astype(np.float32)          # [64, S]
    sinT = np.sin(fr).T.astype(np.float32)
    cos2 = np.ascontiguousarray(np.concatenate([cosT, cosT], axis=0))
    sin2 = np.ascontiguousarray(np.concatenate([-sinT, sinT], axis=0))

    # triT[i, j] = 0 if i <= j else NEG   (valid iff query >= key)
    triT = np.where(np.arange(P)[:, None] <= np.arange(P)[None, :],
                    0.0, NEG).astype(np.float32)
    ident = np.eye(P, dtype=np.float32).astype(ml_dtypes.bfloat16)
    id32 = np.eye(P, dtype=np.float32)
    pswap = np.zeros((P, P), dtype=np.float32)
    pswap[(np.arange(P) + 64) % P, np.arange(P)] = 1.0
    oneh = np.zeros((NBLK, NBLK, P), dtype=np.float32)
    for b_ in range(NBLK):
        oneh[b_, b_, :] = 1.0
    oneh = oneh.reshape(NBLK, NBLK * P)

    mulM = np.ones((P, P), dtype=np.float32)
    addM = np.zeros((P, P), dtype=np.float32)
    for t in range(NT):
        bq = t // 2
        for n in range(NBLK):
            col = t * NBLK + n
            if n == bq:
                mulM[:, col] = 0.0
                addM[:, col] = POS
            elif n > bq:
                addM[:, col] = NEG

    wtile = np.concatenate([w, w])                  # [256]
    in_maps = []
    for c in range(NCORES):
        hs = slice(FLOC * c, FLOC * (c + 1))
        wq_c = np.ascontiguousarray(Wq[hs, :].T)    # [D, 256]
        wk_c = np.ascontiguousarray(Wk[hs, :].T)
        wv_c = np.ascontiguousarray(Wv[hs, :].T)
        wo_c = np.ascontiguousarray((Wo[:, hs] * wtile[None, :]).T).astype(
            ml_dtypes.bfloat16)                     # [256, D]
        in_maps.append({
            "hT": hT, "wq": wq_c, "wk": wk_c, "wv": wv_c, "wo": wo_c,
            "cos2": cos2, "sin2": sin2, "triT": triT, "ident": ident,
            "id32": id32, "pswap": pswap, "oneh": oneh,
            "mulM": mulM, "addM": addM,
        })
    return in_maps


def get_program():
    if "nc" not in _CACHE:
        _CACHE["nc"] = _build_program()
    return _CACHE["nc"]


def run(inputs, trace=False):
    """Returns (output [1,S,D] float32, BassKernelResults)."""
    from concourse import bass_utils

    in_maps = _host_inputs(
        inputs["hidden_states"], inputs["Wq"], inputs["Wk"],
        inputs["Wv"], inputs["Wo"], inputs["o_norm_w"])
    nc = get_program()
    res = bass_utils.run_bass_kernel_spmd(
        nc, in_maps, core_ids=list(range(NCORES)), trace=trace)
    acc = np.zeros((S, D), dtype=np.float32)
    for r in res.results:
        acc += np.asarray(r["out"], dtype=np.float32)
    return acc.reshape(1, S, D), res


def kernel(**inputs):
    out, _ = run(inputs, trace=False)
    return out



# revision 52
# speedup vs baseline: 1.0925x; 1.0925x over previous
"""MoBA (Mixture of Block Attention) Trainium2 Bass kernel — streaming.

Problem: B=1, S=2048, D=2048, H=16 heads (d=128), chunk=256, topk=4.
Sharding: 2 heads per core across 8 cores (tensor-parallel on H).

Single software-pipelined loop over eight 256-token strips.  Strip st:
q/k/v projections (fp32r), RoPE, incremental block-key-sums, per-query
gate + top-4 block mask; then the attention for strip st-1 (scoresT ->
masked exp -> probsT -> PV in transposed-V layout producing OT[f,q] ->
feat-major RMSNorm -> partial output projection).  The lag-1 pipeline
keeps the TensorEngine fed while DVE does top-k and Act does exp.

Key structure:
 - scoresT[k, q] per key-chunk pair packed two chunks to a PSUM bank;
   block mask rows added via an 8-partition one-hot matmul, the causal
   triangle / future-half NEG via identity matmuls (all PE, no DVE in
   the exp dependency chain).
 - PV uses lhsT=V so the context lands transposed (OT[f, q]); RMSNorm
   is then feat-major: sumsq by ones-matmul, rsqrt as exp(-0.5*ln x)
   (ln+exp live in one Act table: no table reloads), scale broadcast
   by a 1-partition outer-product matmul.  No PE transposes of O.
 - fp16 partial-output stores (host sums 8 cores in fp32).

Self-contained: hardcodes all shapes; builds and caches one Bass/Tile
program, runs it SPMD on cores 0-7 via run_bass_kernel_spmd.
"""

import math
from contextlib import ExitStack

import numpy as np
import ml_dtypes

S = 2048
D = 2048
H = 16
DH = 128          # head dim
CHUNK = 256
NBLK = S // CHUNK  # 8
NCORES = 8
HLOC = H // NCORES  # 2 heads per core
FLOC = HLOC * DH    # 256 features per core
P = 128
NT = S // P         # 16 query tiles of 128
NST = S // CHUNK    # 8 strips of 256
SM_SCALE = DH ** -0.5
NEG = -1.0e30
POS = 1.0e30
NEGBIG = -3.0e30
EPS = 1e-6
THETA = 10000.0
SS_SCALE = 256.0    # norm scale premultiplier, folded out of wo on host

SPIN_MMS = 24       # PE warm-up matmuls to ramp the p-state clock

_CACHE = {}


def _build_program(debug=False):
    import concourse.bacc as bacc
    import concourse.tile as tile
    from concourse import mybir

    f32 = mybir.dt.float32
    f32r = mybir.dt.float32r
    bf16 = mybir.dt.bfloat16
    fp16 = mybir.dt.float16
    X = mybir.AxisListType
    AF = mybir.ActivationFunctionType
    OP = mybir.AluOpType

    nc = bacc.Bacc("TRN2", target_bir_lowering=False, debug=False)

    def din(name, shape, dt):
        return nc.dram_tensor(name, shape, dt, kind="ExternalInput").ap()

    hT_d = din("hT", [D, S], f32r)          # hidden transposed [D, S]
    wq_d = din("wq", [D, FLOC], f32r)       # Wq[hs,:].T
    wk_d = din("wk", [D, FLOC], f32r)
    wv_d = din("wv", [D, FLOC], f32r)
    wo_d = din("wo", [FLOC, D], bf16)       # (Wo[:,hs] * w / SS_SCALE).T
    cos2_d = din("cos2", [P, S], f32)       # [cosT; cosT]
    sin2_d = din("sin2", [P, S], f32)       # [-sinT; sinT]
    triT_d = din("triT", [P, P], bf16)      # 0 if k<=q else NEG (within chunk)
    id_d = din("ident", [P, P], bf16)       # identity (bf16)
    idh_d = din("identh", [P, P], fp16)     # identity (fp16)
    pswap_d = din("pswap", [P, P], f32r)    # half-swap permutation
    oneh_d = din("oneh", [NBLK, NBLK * P], bf16)  # oneh[n, b*128+m] = (n==b)
    out_d = nc.dram_tensor("out", [S, D], fp16, kind="ExternalOutput").ap()
    dbg = {}
    if debug:
        for nm, shape, dt in [
                ("dbg_kT", [P, HLOC, S], f32), ("dbg_km", [P, HLOC, NBLK], f32),
                ("dbg_qT3", [P, HLOC, 256], f32), ("dbg_pb3", [P, HLOC, NT, 256], bf16),
                ("dbg_aux3", [P, 8], f32), ("dbg_sS43", [P, 4], fp16),
                ("dbg_onT3", [P, HLOC, 2, P], bf16),
                ("dbg_gate4", [P, 4, NBLK], f32), ("dbg_M4", [P, 4, NBLK], bf16),
                ("dbg_v", [P, NT, FLOC], bf16)]:
            dbg[nm] = nc.dram_tensor(nm, shape, dt, kind="ExternalOutput").ap()

    hT_r = hT_d.rearrange("(dc p) s -> p dc s", p=P)

    with tile.TileContext(nc) as tc:
        with ExitStack() as ctx:
            const_pool = ctx.enter_context(tc.tile_pool(name="consts", bufs=1))
            w_pool = ctx.enter_context(tc.tile_pool(name="weights", bufs=1))
            ht_pool = ctx.enter_context(tc.tile_pool(name="hid", bufs=2))
            act_pool = ctx.enter_context(tc.tile_pool(name="acts", bufs=1))
            strip_pool = ctx.enter_context(tc.tile_pool(name="strip", bufs=2))
            ps_big = ctx.enter_context(
                tc.tile_pool(name="psbig", bufs=6, space="PSUM"))
            ps_ot = ctx.enter_context(
                tc.tile_pool(name="psot", bufs=1, space="PSUM"))
            ps_aux = ctx.enter_context(
                tc.tile_pool(name="psaux", bufs=1, space="PSUM"))

            # ---- critical-path DMAs, single sync queue in priority order
            # (k-projection is emitted first, so wk/ht0 lead) ----
            wq_t = w_pool.tile([P, 16, FLOC], f32r)
            wk_t = w_pool.tile([P, 16, FLOC], f32r)
            wv_t = w_pool.tile([P, 16, FLOC], f32r)
            cos2_t = const_pool.tile([P, S], f32)
            sin2_t = const_pool.tile([P, S], f32)
            pswap_t = const_pool.tile([P, P], f32r)
            triT_t = const_pool.tile([P, P], bf16)
            id_t = const_pool.tile([P, P], bf16)
            wo_t = const_pool.tile([P, HLOC, D], bf16)
            oneh_t = const_pool.tile([NBLK, NBLK, P], bf16)
            wk_r = wk_d.rearrange("(dc p) f -> p dc f", p=P)
            ht0 = ht_pool.tile([P, 16, 256], f32r, tag="hid", name="ht0")
            nc.sync.dma_start(wk_t[:, 0:8, :], wk_r[:, 0:8, :])
            nc.sync.dma_start(ht0[:, 0:8, :], hT_r[:, 0:8, 0:256])
            nc.sync.dma_start(wk_t[:, 8:16, :], wk_r[:, 8:16, :])
            nc.sync.dma_start(ht0[:, 8:16, :], hT_r[:, 8:16, 0:256])
            nc.sync.dma_start(wq_t[:], wq_d.rearrange("(dc p) f -> p dc f", p=P))
            nc.sync.dma_start(pswap_t[:], pswap_d)
            nc.sync.dma_start(cos2_t[:], cos2_d)
            nc.sync.dma_start(sin2_t[:], sin2_d)
            nc.sync.dma_start(wv_t[:], wv_d.rearrange("(dc p) f -> p dc f", p=P))
            nc.sync.dma_start(triT_t[:], triT_d)
            # bulky late-use constants ride the gpsimd queue (round-robins
            # with sync, stealing only a small share of early bus slots)
            idh_t = const_pool.tile([P, P], fp16)
            # (id/idh/oneh/wo are loaded later, after ht1, on the sync queue)

            # ---- on-chip constants ----
            zero_t = const_pool.tile([P, 1], f32)
            onescol_t = const_pool.tile([P, 1], bf16)
            onesrow_t = const_pool.tile([1, P], fp16)
            negfull_t = const_pool.tile([P, P], bf16)
            negbig_t = const_pool.tile([P, 4 * NBLK], f32)
            spin_t = const_pool.tile([P, 512], bf16)
            magic_t = const_pool.tile([P, 1], f32)
            # float whose bits are 0x5f3759df (fast inverse sqrt seed)
            nc.vector.memset(
                magic_t[:],
                float(np.uint32(0x5F3759DF).view(np.float32)))
            nc.vector.memset(zero_t[:], 0.0)
            nc.vector.memset(onescol_t[:], 1.0)
            nc.vector.memset(onesrow_t[:], 1.0)
            nc.vector.memset(negfull_t[:], NEG)
            nc.vector.memset(negbig_t[:], NEGBIG)
            nc.vector.memset(spin_t[:], 0.5)

            # PE p-state warm-up: keep the array busy while DMAs stream so
            # the clock is at 2.4 GHz when the first projection lands.
            for _ in range(SPIN_MMS):
                psp = ps_big.tile([P, 512], f32, tag="big", name="psp")
                nc.tensor.matmul(psp[:], lhsT=spin_t[:, 0:P], rhs=spin_t[:],
                                 start=True, stop=True)

            # ---- persistent activations ----
            kT = act_pool.tile([P, HLOC, S], f32r)      # roped k, [d, h, s]
            v_sb = act_pool.tile([P, NT, FLOC], bf16)   # [tok, chunk, feat]
            km_t = act_pool.tile([P, HLOC, NBLK], f32r)
            # future blocks read as finite 0 (f32r rejects memset; bitcast)
            nc.vector.memset(km_t[:].bitcast(mybir.dt.uint32), 0)

            def strip_tiles(st):
                """Per-strip ring tiles (bufs=2 via strip_pool)."""
                d = {}
                d["qT"] = strip_pool.tile([P, HLOC, 256], f32r, tag="qT",
                                          name=f"qT{st}")
                d["pb"] = strip_pool.tile([P, HLOC, NT, 256], bf16, tag="pb",
                                          name=f"pb{st}")
                d["gate"] = strip_pool.tile([P, 4, NBLK], f32, tag="gate",
                                            name=f"gate{st}")
                d["gw"] = strip_pool.tile([P, 4, NBLK], f32, tag="gw",
                                          name=f"gw{st}")
                d["lt"] = strip_pool.tile([P, 4, NBLK], mybir.dt.int32,
                                          tag="lt", name=f"lt{st}")
                d["m4"] = strip_pool.tile([P, 4], f32, tag="m4", name=f"m4{st}")
                d["M"] = strip_pool.tile([P, 4, NBLK], bf16, tag="M",
                                         name=f"M{st}")
                d["MTf"] = strip_pool.tile([NBLK, HLOC, 256], bf16, tag="MTf",
                                           name=f"MTf{st}")
                d["sq"] = strip_pool.tile([P, 4, P], bf16, tag="sq",
                                          name=f"sq{st}")
                d["n1"] = strip_pool.tile([P, 4], f32, tag="n1",
                                          name=f"n1{st}")
                d["n2"] = strip_pool.tile([P, 4], f32, tag="n2",
                                          name=f"n2{st}")
                d["n3"] = strip_pool.tile([P, 4], f32, tag="n3",
                                          name=f"n3{st}")
                d["sS4"] = strip_pool.tile([P, 4], fp16, tag="sS4",
                                           name=f"sS4{st}")
                d["sST"] = strip_pool.tile([1, 4, P], fp16, tag="sST",
                                           name=f"sST{st}")
                d["bcsb"] = strip_pool.tile([P, 4, P], f32, tag="bcsb",
                                            name=f"bcsb{st}")
                d["onT"] = strip_pool.tile([P, HLOC, 2, P], bf16, tag="onT",
                                           name=f"onT{st}")
                return d

            ST = [None] * NST
            ATT = {}  # st -> dict with psum handles shared across att parts

            def emit_proj(st):
                """Projections+rope+km+gate+mask for strip st; prefetch st+1."""
                sl = slice(st * 256, (st + 1) * 256)
                ht = ht0 if st == 0 else ST[st]["ht"]
                d = strip_tiles(st)
                d["ht"] = ht
                ST[st] = d
                if st + 1 < NST:
                    htn = ht_pool.tile([P, 16, 256], f32r, tag="hid",
                                       name=f"ht{st+1}")
                    nc.sync.dma_start(htn[:], hT_r[:, :, slice((st + 1) * 256,
                                                               (st + 2) * 256)])
                    ST[st + 1] = {"ht": htn}  # stash for the next strip
                qT_s = d["qT"]

                # k projection: both head-halves into one [128,512] psum
                pk = ps_big.tile([P, 512], f32, tag="big", name="pk")
                for fc in range(HLOC):
                    for dc in range(16):
                        nc.tensor.matmul(
                            pk[:, fc * 256:(fc + 1) * 256],
                            lhsT=wk_t[:, dc, fc * P:(fc + 1) * P],
                            rhs=ht[:, dc, :],
                            start=(dc == 0), stop=(dc == 15))
                nc.vector.tensor_copy(
                    kT[:, :, sl], pk[:].rearrange("p (h s) -> p h s", h=HLOC))
                # q projection
                pq = ps_big.tile([P, 512], f32, tag="big", name="pq")
                for fc in range(HLOC):
                    for dc in range(16):
                        nc.tensor.matmul(
                            pq[:, fc * 256:(fc + 1) * 256],
                            lhsT=wq_t[:, dc, fc * P:(fc + 1) * P],
                            rhs=ht[:, dc, :],
                            start=(dc == 0), stop=(dc == 15))
                nc.vector.tensor_copy(
                    qT_s[:], pq[:].rearrange("p (h s) -> p h s", h=HLOC))

                # rope k: psum swap-mm then 3 DVE ops over [128, 2, 256]
                cosb = cos2_t[:, sl].rearrange("p (o s) -> p o s", o=1)\
                    .to_broadcast((P, HLOC, 256))
                sinb = sin2_t[:, sl].rearrange("p (o s) -> p o s", o=1)\
                    .to_broadcast((P, HLOC, 256))
                for dst, view in ((kT, kT[:, :, sl]), (qT_s, qT_s[:])):
                    psw = ps_big.tile([P, 512], f32, tag="big", name="psw")
                    for hh in range(HLOC):
                        src = (kT[:, hh, sl] if dst is kT else qT_s[:, hh, :])
                        nc.tensor.matmul(
                            psw[:, hh * 256:(hh + 1) * 256],
                            lhsT=pswap_t[:], rhs=src,
                            start=True, stop=True)
                    pswv = psw[:].rearrange("p (h s) -> p h s", h=HLOC)
                    scr = strip_pool.tile([P, HLOC, 256], f32, tag="ropescr",
                                          name="scr")
                    nc.vector.tensor_mul(scr[:], pswv, sinb)
                    nc.vector.tensor_mul(view, view, cosb)
                    nc.vector.tensor_add(view, view, scr[:])
                # block key sums (mean scaling irrelevant to top-k order)
                with nc.allow_low_precision(reason="km read in fp32r gate"):
                    nc.vector.reduce_sum(km_t[:, :, st:st + 1], kT[:, :, sl],
                                         axis=X.X)

                # v projection
                pv = ps_big.tile([P, 512], f32, tag="big", name="pv")
                for sc in range(2):
                    for dc in range(16):
                        nc.tensor.matmul(
                            pv[:, sc * 256:(sc + 1) * 256],
                            lhsT=ht[:, dc, sc * P:(sc + 1) * P],
                            rhs=wv_t[:, dc, :],
                            start=(dc == 0), stop=(dc == 15))
                nc.vector.tensor_copy(
                    v_sb[:, 2 * st:2 * st + 2, :],
                    pv[:].rearrange("p (c f) -> p c f", c=2))

            def emit_gate(st):
                """Gate + top-4 mask for strip st (emitted mid-attention of
                strip st-1 so DVE has finished rope+km by then)."""
                if st < 4:
                    return  # all causal blocks selected; no gate/mask needed
                d = ST[st]
                qT_s = d["qT"]

                # gate: full 8-wide matmul (ISA rejects odd moving dims);
                # future-block columns masked NEG after the copy
                pg = ps_big.tile([P, 4 * NBLK], f32, tag="big", name="pg")
                for hh in range(HLOC):
                    for tl in range(2):
                        g = hh * 2 + tl
                        nc.tensor.matmul(
                            pg[:, g * NBLK:(g + 1) * NBLK],
                            lhsT=qT_s[:, hh, tl * P:(tl + 1) * P],
                            rhs=km_t[:, hh, :],
                            start=True, stop=True)
                gate_s = d["gate"]
                nc.vector.tensor_copy(
                    gate_s[:].rearrange("p g n -> p (g n)"), pg[:])
                if st + 1 < NBLK:
                    nc.vector.memset(gate_s[:, :, st + 1:], NEG)  # future
                nc.vector.memset(gate_s[:, :, st:st + 1], POS)  # self block

                # top-4: suppress max 3x, threshold on 4th max
                gw, lt, m4, M_s = d["gw"], d["lt"], d["m4"], d["M"]
                gv = gate_s[:]
                gwv = gw[:]
                ltv = lt[:]
                nc.vector.tensor_copy(gwv, gv)
                for _ in range(3):
                    nc.vector.reduce_max(m4[:], gwv, axis=X.X)
                    m4b = m4[:].rearrange("p (g o) -> p g o", o=1)\
                        .to_broadcast((P, 4, NBLK))
                    nc.vector.tensor_tensor(ltv, gwv, m4b, op=OP.is_ge)
                    nc.vector.copy_predicated(
                        gwv, ltv,
                        negbig_t[:].rearrange("p (g n) -> p g n", g=4))
                nc.vector.reduce_max(m4[:], gwv, axis=X.X)
                m4b = m4[:].rearrange("p (g o) -> p g o", o=1)\
                    .to_broadcast((P, 4, NBLK))
                nc.vector.tensor_tensor(M_s[:], gv, m4b, op=OP.is_ge)
                nc.vector.tensor_scalar(
                    M_s[:], M_s[:], 1.0, POS, op0=OP.subtract, op1=OP.mult)
                if debug and st == 4:
                    nc.sync.dma_start(dbg["dbg_gate4"], gate_s[:])
                    nc.sync.dma_start(dbg["dbg_M4"], M_s[:])

                # transpose mask rows -> MTf[n, hh, 128*tl + p]
                MTf = d["MTf"]
                pmt = ps_big.tile([NBLK, 512], bf16, tag="big", name="pmt")
                for hh in range(HLOC):
                    for tl in range(2):
                        g = hh * 2 + tl
                        nc.tensor.transpose(
                            pmt[:, g * P:(g + 1) * P], M_s[:, g, :], id_t[:])
                        nc.vector.tensor_copy(
                            MTf[:, hh, tl * P:(tl + 1) * P],
                            pmt[:, g * P:(g + 1) * P])

            def emit_wo(s, tl):
                """Output projection + store for query tile t=2s+tl."""
                d = ST[s]
                t = 2 * s + tl
                orow = strip_pool.tile([P, D], fp16, tag="orow", name="orow")
                for nt in range(4):
                    pso = ps_big.tile([P, 512], f32, tag="big", name="pso")
                    for hh in range(HLOC):
                        nc.tensor.matmul(
                            pso[:],
                            lhsT=d["onT"][:, hh, tl, :],
                            rhs=wo_t[:, hh, nt * 512:(nt + 1) * 512],
                            start=(hh == 0), stop=(hh == HLOC - 1))
                    if nt % 2 == 0:
                        nc.vector.tensor_copy(
                            orow[:, nt * 512:(nt + 1) * 512], pso[:])
                    else:
                        nc.scalar.copy(
                            orow[:, nt * 512:(nt + 1) * 512], pso[:])
                nc.gpsimd.dma_start(out_d[t * P:(t + 1) * P, :], orow[:])

            def emit_att(s):
                """Attention for strip s (queries 256s..256s+255)."""
                d = ST[s]
                qT_s, pb, MTf = d["qT"], d["pb"], d["MTf"]

                # finish the previous strip's norm (its DVE rsqrt chain ran
                # during the projection block emitted between the two atts)
                if s > 0:
                    emit_norm_finish(s - 1)

                # scoresT per chunk-pair, packed 2 chunks to a psum bank
                for hh in range(HLOC):
                    for j in range(s + 1):
                        psc = ps_big.tile([P, 512], f32, tag="big", name="psc")
                        if j < s:
                            # full pair: chunks 2j, 2j+1 x 256 queries
                            for ci in range(2):
                                c = 2 * j + ci
                                co = ci * 256
                                nc.tensor.matmul(
                                    psc[:, co:co + 256],
                                    lhsT=kT[:, hh, c * P:(c + 1) * P],
                                    rhs=qT_s[:, hh, :],
                                    start=True, stop=(s < 4),
                                    skip_group_check=True)
                                if s >= 4:
                                    nc.tensor.matmul(
                                        psc[:, co:co + 256],
                                        lhsT=oneh_t[:, j, :],
                                        rhs=MTf[:, hh, :],
                                        start=False, stop=True,
                                        skip_group_check=True)
                        else:
                            # self pair: chunks 2s (diag at tl=0), 2s+1
                            for ci in range(2):
                                c = 2 * s + ci
                                co = ci * 256
                                nc.tensor.matmul(
                                    psc[:, co:co + 256],
                                    lhsT=kT[:, hh, c * P:(c + 1) * P],
                                    rhs=qT_s[:, hh, :],
                                    start=True, stop=False,
                                    skip_group_check=True)
                                # causal triangle on the diagonal 128-block
                                nc.tensor.matmul(
                                    psc[:, co + ci * P:co + ci * P + P],
                                    lhsT=id_t[:],
                                    rhs=triT_t[:],
                                    start=False, stop=(ci == 0),
                                    skip_group_check=True)
                                if ci == 1:
                                    # chunk 2s+1 vs q-tile 2s: all future
                                    nc.tensor.matmul(
                                        psc[:, co:co + P],
                                        lhsT=id_t[:],
                                        rhs=negfull_t[:],
                                        start=False, stop=True,
                                        skip_group_check=True)
                        nc.scalar.activation(
                            pb[:, hh, 2 * j:2 * j + 2, :].rearrange(
                                "p c q -> p (c q)"),
                            psc[:], AF.Exp, bias=zero_t[:], scale=SM_SCALE)

                if debug and s == 3:
                    nc.sync.dma_start(dbg["dbg_pb3"], pb[:])
                    nc.sync.dma_start(dbg["dbg_qT3"],
                                      qT_s[:].bitcast(mybir.dt.float32))

                # next strip's gate: DVE rope/km for s+1 are done by now
                if s + 1 < NST:
                    emit_gate(s + 1)
                # previous strip's first WO tile (onT ready: bc-mm + DVE mult
                # ran during the scores above)
                if s > 0:
                    emit_wo(s - 1, 0)

                # PV: OT[f, q] and row-sums, per (hh, tloc)
                po = ps_ot.tile([P, 512], f32, tag="ot", name="po")
                aux = ps_aux.tile([P, 512], f32, tag="aux", name="aux")
                sq = d["sq"]
                for hh in range(HLOC):
                    for tl in range(2):
                        g = hh * 2 + tl
                        t = 2 * s + tl
                        for c2 in range(t + 1):
                            pbv = pb[:, hh, c2, tl * P:(tl + 1) * P]
                            nc.tensor.matmul(
                                po[:, g * P:(g + 1) * P],
                                lhsT=v_sb[:, c2, hh * DH:(hh + 1) * DH],
                                rhs=pbv,
                                start=(c2 == 0), stop=(c2 == t))
                            # row-sums land q-major: [128q, 1] column
                            nc.tensor.matmul(
                                aux[:, g:g + 1],
                                lhsT=pbv, rhs=onescol_t[:],
                                start=(c2 == 0), stop=(c2 == t))
                        nc.scalar.activation(
                            sq[:, g, :], po[:, g * P:(g + 1) * P], AF.Square)

                # sumsq q-major via ones-rhs matmul
                for g in range(4):
                    nc.tensor.matmul(
                        aux[:, 4 + g:5 + g],
                        lhsT=sq[:, g, :], rhs=onescol_t[:],
                        start=True, stop=True)

                # second WO tile of previous strip covers the norm latency
                if s > 0:
                    emit_wo(s - 1, 1)

                # x = (sumsq/DH + (r*sqrt(EPS))^2) / SS_SCALE^2, then
                # sS = rsqrt(x) = SS_SCALE/sqrt(x_true): bit-trick + 2 Newton
                n1, n2, n3 = d["n1"], d["n2"], d["n3"]
                sS4, sST = d["sS4"], d["sST"]
                magic_b = magic_t[:].to_broadcast((P, 4))
                nc.vector.tensor_scalar_mul(n1[:], aux[:, 0:4],
                                            math.sqrt(EPS) / SS_SCALE)
                nc.vector.tensor_tensor(n1[:], n1[:], n1[:], op=OP.mult)
                nc.vector.tensor_scalar_mul(n2[:], aux[:, 4:8],
                                            1.0 / (DH * SS_SCALE * SS_SCALE))
                nc.vector.tensor_add(n2[:], n2[:], n1[:])
                if debug and s == 3:
                    dbga = const_pool.tile([P, 8], f32, name="dbga")
                    nc.vector.tensor_copy(dbga[:], aux[:, 0:8])
                    nc.sync.dma_start(dbg["dbg_aux3"], dbga[:])
                i32 = mybir.dt.int32
                nc.vector.tensor_scalar(
                    n3[:].bitcast(i32), n2[:].bitcast(i32), 1, None,
                    op0=OP.logical_shift_right)
                nc.vector.tensor_tensor(
                    n3[:].bitcast(i32), magic_b.bitcast(i32),
                    n3[:].bitcast(i32), op=OP.subtract)
                for it in range(2):
                    nc.vector.tensor_tensor(n1[:], n2[:], n3[:], op=OP.mult)
                    nc.vector.tensor_tensor(n1[:], n1[:], n3[:], op=OP.mult)
                    nc.vector.tensor_scalar(n1[:], n1[:], -0.5, 1.5,
                                            op0=OP.mult, op1=OP.add)
                    if it == 0:
                        nc.vector.tensor_tensor(n3[:], n3[:], n1[:],
                                                op=OP.mult)
                    else:
                        nc.vector.tensor_tensor(sS4[:], n3[:], n1[:],
                                                op=OP.mult)
                if debug and s == 3:
                    nc.sync.dma_start(dbg["dbg_sS43"], sS4[:])

                ATT[s] = po

            def emit_norm_finish(s):
                """Scale broadcast + onT for strip s — emitted a block later
                so the DVE rsqrt chain finishes without stalling the PE."""
                d = ST[s]
                sS4, sST = d["sS4"], d["sST"]
                po = ATT.pop(s)
                # transpose each scale column to a [1,128] row, then
                # broadcast across partitions via ones-row outer products
                pst = ps_big.tile([1, 4, P], fp16, tag="big", name="pst")
                for g in range(4):
                    nc.tensor.transpose(pst[0:1, g, :], sS4[:, g:g + 1],
                                        idh_t[:])
                nc.vector.tensor_copy(sST[:], pst[:])
                pbc = ps_big.tile([P, 512], f32, tag="big", name="pbc")
                for g in range(4):
                    nc.tensor.matmul(
                        pbc[:, g * P:(g + 1) * P],
                        lhsT=onesrow_t[:], rhs=sST[0:1, g, :],
                        start=True, stop=True)
                # DVE tensor_tensor may read only one PSUM operand: stage
                # the broadcast through SBUF on the scalar engine
                bc_sb = d["bcsb"]
                nc.scalar.copy(bc_sb[:].rearrange("p g q -> p (g q)"), pbc[:])
                nc.vector.tensor_tensor(
                    d["onT"][:].rearrange("p h t q -> p (h t q)"),
                    po[:], bc_sb[:].rearrange("p g q -> p (g q)"), op=OP.mult)
                if debug and s == 3:
                    nc.sync.dma_start(dbg["dbg_onT3"], d["onT"][:])

            for st in range(NST):
                emit_proj(st)
                if st == 0:
                    # late-use constants: queued on sync after ht1 so they
                    # don't steal early bus slots from the critical loads
                    nc.sync.dma_start(id_t[:], id_d)
                    nc.sync.dma_start(idh_t[:], idh_d)
                    nc.sync.dma_start(
                        oneh_t[:], oneh_d.rearrange("k (b m) -> k b m", m=P))
                    nc.sync.dma_start(
                        wo_t[:], wo_d.rearrange("(fc p) j -> p fc j", p=P))
                if st >= 1:
                    emit_att(st - 1)
            emit_att(NST - 1)
            emit_norm_finish(NST - 1)
            emit_wo(NST - 1, 0)
            emit_wo(NST - 1, 1)
            if debug:
                nc.sync.dma_start(dbg["dbg_kT"],
                                  kT[:].bitcast(mybir.dt.float32))
                nc.sync.dma_start(dbg["dbg_km"],
                                  km_t[:].bitcast(mybir.dt.float32))
                nc.sync.dma_start(dbg["dbg_v"], v_sb[:])

    nc.compile()
    return nc


def _host_inputs(hidden, Wq, Wk, Wv, Wo, o_norm_w):
    """Build the per-core input maps (host-side sharding + prep)."""
    def fp22_round(x):
        """Round fp32 mantissa to 13 bits (FP22, round-half-to-even) so the
        fp32r TensorEngine path sees exactly these values."""
        u = np.ascontiguousarray(x, dtype=np.float32).view(np.uint32)
        lsb = (u >> np.uint32(10)) & np.uint32(1)
        r = (u + np.uint32(0x1FF) + lsb) & np.uint32(0xFFFFFC00)
        return r.view(np.float32)

    h = np.ascontiguousarray(np.asarray(hidden, dtype=np.float32).reshape(S, D))
    Wq = fp22_round(np.asarray(Wq, dtype=np.float32))
    Wk = fp22_round(np.asarray(Wk, dtype=np.float32))
    Wv = fp22_round(np.asarray(Wv, dtype=np.float32))
    Wo = np.asarray(Wo, dtype=np.float32)
    w = np.asarray(o_norm_w, dtype=np.float32)

    hT = fp22_round(np.ascontiguousarray(h.T))

    pos = np.arange(S, dtype=np.float64)
    inv = 1.0 / (THETA ** (np.arange(0, DH, 2, dtype=np.float64) / DH))
    fr = pos[:, None] * inv[None, :]                # [S, 64]
    cosT = np.cos(fr).T.astype(np.float32)          # [64, S]
    sinT = np.sin(fr).T.astype(np.float32)
    cos2 = np.ascontiguousarray(np.concatenate([cosT, cosT], axis=0))
    sin2 = np.ascontiguousarray(np.concatenate([-sinT, sinT], axis=0))

    # triT[k, q] = 0 if k <= q else NEG  (valid iff query >= key)
    triT = np.where(np.arange(P)[:, None] <= np.arange(P)[None, :],
                    0.0, NEG).astype(ml_dtypes.bfloat16)
    ident = np.eye(P, dtype=np.float32).astype(ml_dtypes.bfloat16)
    identh = np.eye(P, dtype=np.float32).astype(np.float16)
    pswap = np.zeros((P, P), dtype=np.float32)
    pswap[(np.arange(P) + 64) % P, np.arange(P)] = 1.0
    oneh = np.zeros((NBLK, NBLK, P), dtype=np.float32)
    for b_ in range(NBLK):
        oneh[b_, b_, :] = 1.0
    oneh = oneh.reshape(NBLK, NBLK * P).astype(ml_dtypes.bfloat16)

    wtile = np.concatenate([w, w])                  # [256]
    in_maps = []
    for c in range(NCORES):
        hs = slice(FLOC * c, FLOC * (c + 1))
        wq_c = np.ascontiguousarray(Wq[hs, :].T)    # [D, 256]
        wk_c = np.ascontiguousarray(Wk[hs, :].T)
        wv_c = np.ascontiguousarray(Wv[hs, :].T)
        wo_c = np.ascontiguousarray(
            (Wo[:, hs] * (wtile[None, :] / SS_SCALE)).T).astype(
            ml_dtypes.bfloat16)                     # [256, D]
        in_maps.append({
            "hT": hT, "wq": wq_c, "wk": wk_c, "wv": wv_c, "wo": wo_c,
            "cos2": cos2, "sin2": sin2, "triT": triT, "ident": ident,
            "identh": identh, "pswap": pswap, "oneh": oneh,
        })
    return in_maps


def get_program():
    if "nc" not in _CACHE:
        _CACHE["nc"] = _build_program()
    return _CACHE["nc"]


def run(inputs, trace=False):
    """Returns (output [1,S,D] float32, BassKernelResults)."""
    from concourse import bass_utils

    in_maps = _host_inputs(
        inputs["hidden_states"], inputs["Wq"], inputs["Wk"],
        inputs["Wv"], inputs["Wo"], inputs["o_norm_w"])
    nc = get_program()
    res = bass_utils.run_bass_kernel_spmd(
        nc, in_maps, core_ids=list(range(NCORES)), trace=trace)
    acc = np.zeros((S, D), dtype=np.float32)
    for r in res.results:
        acc += np.asarray(r["out"], dtype=np.float32)
    return acc.reshape(1, S, D), res


def kernel(**inputs):
    out, _ = run(inputs, trace=False)
    return out
